# revision 6
# baseline (speedup 1.0000x reference)
"""Trainium2 Bass kernel for the contrastive loss problem.

Math reformulation of the reference (no [N, 2N-1] scatter needed):
  lse_i = log( exp(pos_val_i) + sum_{j in neg} exp(S_ij) + (2N-2-num_neg_i) )
  loss  = mean_i (lse_i - pos_val_i)
with S = (cos + 1) * 0.25, cos from row-normalized embeddings.

Sharding uses the Gram matrix's symmetry: core c computes only the
[512, 512*5] strip of exp(S) pairing its rows with block-columns
{c, c+1, .., c+4} (mod 8). Columns are pre-rotated on the host so the
program is identical on every core (SPMD). Row sums cover the strip;
one-hot-weight matmuls produce per-column sums for the foreign blocks
(distance 1..3), which the host adds to those rows' totals. Distance-4
blocks are computed by both endpoint cores (row sums only). The main
matmul runs in fp8 e4m3 (DoubleRow, K=256 per op) on x16-prescaled unit
rows. exp/masking on ScalarE/VectorE per 512-wide chunk; exp output is
stored fp8 and stacked across m-chunk pairs so the column-sum matmuls
are DoubleRow too (12 ops total, one PSUM bank, partitions 0:6).
Input DMAs are split across both HW DGE queues (sync + scalar) to
halve delivery latency; pos-pair row dots run on the idle Pool engine.

Host: norms, fp8/bf16 casts, rotation, first-positive gather (label
metadata), final assembly of ~4096 scalars.
"""

import sys

sys.path.insert(0, "/opt/trn_rl_repo")

from contextlib import ExitStack

import ml_dtypes
import numpy as np

import concourse.bacc as bacc
import concourse.tile as tile
from concourse import mybir
from concourse.bass_utils import run_bass_kernel_spmd

N, D = 4096, 1024
NCORES = 8
R = N // NCORES            # 512 rows per core
P = 128                    # partitions
MI = R // P                # 4 row chunks per core
KC = D // P                # 8 contraction chunks
JW = 512                   # j tile width (one PSUM bank)
NB = 5                     # block-columns per core (self + 4 right neighbors)
JCOLS = NB * JW            # 2560
EPS = 1e-8
BF16 = ml_dtypes.bfloat16
FP8 = ml_dtypes.float8_e4m3
SCALE = 16.0

_CACHE = {}


def _build_program():
    nc = bacc.Bacc("TRN2", target_bir_lowering=False, debug=False)
    f32, bf16, fp8 = mybir.dt.float32, mybir.dt.bfloat16, mybir.dt.float8e4
    AF = mybir.ActivationFunctionType
    OP = mybir.AluOpType
    DR = mybir.MatmulPerfMode.DoubleRow

    et_d = nc.dram_tensor("et", [KC, P, JCOLS], fp8, kind="ExternalInput")
    yt_d = nc.dram_tensor("yt", [P, JCOLS], bf16, kind="ExternalInput")
    yb_d = nc.dram_tensor("yb", [P, MI], f32, kind="ExternalInput")
    enef_d = nc.dram_tensor("enef", [MI, P, 2 * D], bf16, kind="ExternalInput")
    ro_d = nc.dram_tensor("rowout", [P, 2 * MI], f32, kind="ExternalOutput")
    cs_d = nc.dram_tensor("csout", [6, JW], f32, kind="ExternalOutput")

    with tile.TileContext(nc) as tc, ExitStack() as ctx:
        const = ctx.enter_context(tc.tile_pool(name="const", bufs=1))
        psum = ctx.enter_context(tc.tile_pool(name="psum", bufs=4, space="PSUM"))
        cspsum = ctx.enter_context(tc.tile_pool(name="cspsum", bufs=1,
                                                space="PSUM"))
        esp = ctx.enter_context(tc.tile_pool(name="esp", bufs=2))
        work = ctx.enter_context(tc.tile_pool(name="work", bufs=2))
        acc = ctx.enter_context(tc.tile_pool(name="acc", bufs=2))

        et = const.tile([P, KC, JCOLS], fp8, tag="et")
        yt = const.tile([P, JCOLS], bf16, tag="yt")
        yb = const.tile([P, MI], f32, tag="yb")
        enef = const.tile([P, MI, 2, D], bf16, tag="enef")
        b025 = const.tile([P, 1], f32, tag="b025")
        nc.vector.memset(b025, 0.25)
        ones = const.tile([P, 1], bf16, tag="ones")
        nc.gpsimd.memset(ones, 1.0)
        wsrc = const.tile([P, P], bf16, tag="wsrc")
        nc.gpsimd.memset(wsrc, 1.0)
        # one-hot DoubleRow weights: wcs[d][:, :, 0:6] has a 1 in column d.
        # 16-wide padding keeps the k-pair stride a multiple of 16 (DR ISA).
        wcs = [const.tile([P, 2, 16], fp8, tag=f"wc{d}", name=f"wc{d}")
               for d in range(6)]
        for d in range(6):
            nc.gpsimd.memset(wcs[d], 0.0)
            nc.gpsimd.memset(wcs[d][:, :, d:d + 1], 1.0)
        rowout = const.tile([P, 2 * MI], f32, tag="rowout")
        nsout = rowout[:, 0:MI]
        pdout = rowout[:, MI:2 * MI]
        cs = cspsum.tile([P, JW], f32, tag="cs")

        # warm the PE clock gate during the initial DMA wait: tiny matmuls
        # into a partition strip the column sums never touch
        for _ in range(8):
            nc.tensor.matmul(
                cs[96:97, 0:P], ones, wsrc, start=True, stop=True,
                tile_position=(0, 96), skip_group_check=True,
            )

        # Input DMAs split across both HW DGE queues, ordered for earliest
        # PE start (k-chunks interleave even/odd across queues).
        for k in range(0, KC, 2):
            nc.sync.dma_start(out=et[:, k, :], in_=et_d[k])
        for k in range(1, KC, 2):
            nc.scalar.dma_start(out=et[:, k, :], in_=et_d[k])
        nc.scalar.dma_start(out=yb, in_=yb_d[:])
        nc.sync.dma_start(out=yt, in_=yt_d[:])
        for m in range(MI):
            nc.sync.dma_start(out=enef[:, m, :, :], in_=enef_d[m])
        # load the Exp table while waiting on DMAs
        warm = const.tile([P, 1], f32, tag="warm")
        nc.scalar.activation(warm, b025, AF.Exp, bias=b025, scale=1.0)

        for mp in range(2):
            es = [esp.tile([P, 2, JW], fp8, tag=f"es{j}", name=f"es{j}")
                  for j in range(NB)]
            mm = [esp.tile([P, 2, JW], fp8, tag=f"mm{j}", name=f"mm{j}")
                  for j in range(NB)]
            for mh in range(2):
                m = 2 * mp + mh
                t1 = acc.tile([P, NB], f32, tag="t1")
                t2 = acc.tile([P, NB], f32, tag="t2")
                for j in range(NB):
                    pt = psum.tile([P, JW], f32, tag="pt")
                    for k2 in range(KC // 2):
                        nc.tensor.matmul(
                            pt,
                            et[:, 2 * k2:2 * k2 + 2, m * P:(m + 1) * P],
                            et[:, 2 * k2:2 * k2 + 2, j * JW:(j + 1) * JW],
                            start=(k2 == 0),
                            stop=(k2 == KC // 2 - 1),
                            perf_mode=DR,
                        )
                    # expS = exp(cos*0.25 + 0.25); t1[:, j] = row-sum
                    nc.scalar.activation(
                        es[j][:, mh, :], pt, AF.Exp, bias=b025,
                        scale=0.25 / (SCALE * SCALE),
                        accum_out=t1[:, j:j + 1],
                    )
                    # t2[:, j] = row-sum((y == y_row) * expS)
                    nc.vector.scalar_tensor_tensor(
                        mm[j][:, mh, :], yt[:, j * JW:(j + 1) * JW],
                        yb[:, m:m + 1], es[j][:, mh, :],
                        op0=OP.is_equal, op1=OP.mult,
                        accum_out=t2[:, j:j + 1],
                    )
                # nsout[:, m] = sum_j(t1 - t2)
                d5 = acc.tile([P, NB], f32, tag="d5")
                nc.vector.scalar_tensor_tensor(
                    d5, t1, 1.0, t2, op0=OP.mult, op1=OP.subtract,
                    accum_out=nsout[:, m:m + 1],
                )
                # pdout[:, m] = row-wise <e_i, e_firstpos(i)>
                pdo = work.tile([P, D], bf16, tag="pdo")
                nc.vector.scalar_tensor_tensor(
                    pdo, enef[:, m, 0, :], 1.0, enef[:, m, 1, :],
                    op0=OP.mult, op1=OP.mult, accum_out=pdout[:, m:m + 1],
                )
            # column sums for the foreign blocks: DoubleRow over the stacked
            # m-chunk pair, one-hot weights land block d's sums in PSUM
            # partition d (es sums rows 0:3, mm sums rows 3:6).
            for jj in range(1, 4):
                nc.tensor.matmul(
                    cs[0:6, :], wcs[jj - 1][:, :, 0:6], es[jj],
                    start=(mp == 0 and jj == 1), stop=False, perf_mode=DR,
                )
            for jj in range(1, 4):
                nc.tensor.matmul(
                    cs[0:6, :], wcs[3 + jj - 1][:, :, 0:6], mm[jj],
                    start=False, stop=(mp == 1 and jj == 3), perf_mode=DR,
                )
        # evict column sums (DMA cannot read PSUM)
        csev = const.tile([P, JW], f32, tag="csev")
        nc.scalar.copy(csev[0:6, :], cs[0:6, :])
        nc.sync.dma_start(out=ro_d[:, :], in_=rowout)
        nc.scalar.dma_start(out=cs_d[:, :], in_=csev[0:6, :])

    nc.compile()
    return nc


def _get_program():
    if "nc" not in _CACHE:
        _CACHE["nc"] = _build_program()
    return _CACHE["nc"]


def _host_prep(layer_embeds, y_true):
    E = np.asarray(layer_embeds, dtype=np.float32)
    y = np.asarray(y_true).astype(np.int32)

    norms = np.maximum(np.linalg.norm(E, axis=1), EPS).astype(np.float32)
    Ehf = E / norms[:, None]
    Eh = Ehf.astype(BF16)
    Eh8T = np.ascontiguousarray((Ehf * SCALE).astype(FP8).T)  # [D, N]

    same = y[:, None] == y[None, :]
    nsame = same.sum(1)
    haspos = nsame > 1
    np.fill_diagonal(same, False)
    fp = np.argmax(same, axis=1)                      # first positive (j order)
    yb16 = y.astype(BF16)

    in_maps = []
    for c in range(NCORES):
        r0, r1 = c * R, (c + 1) * R
        cols = np.concatenate(
            [np.arange(((c + b) % NCORES) * R, ((c + b) % NCORES) * R + R)
             for b in range(NB)])
        etc = np.ascontiguousarray(Eh8T[:, cols]).reshape(KC, P, JCOLS)
        ytc = np.ascontiguousarray(
            np.broadcast_to(yb16[cols][None, :], (P, JCOLS)))
        enc = Eh[r0:r1].reshape(MI, P, D)
        efc = Eh[fp[r0:r1]].reshape(MI, P, D)
        in_maps.append({
            "et": etc,
            "yt": ytc,
            "yb": np.ascontiguousarray(y[r0:r1].astype(np.float32)
                                       .reshape(MI, P).T),
            "enef": np.ascontiguousarray(
                np.concatenate([enc, efc], axis=2)),
        })
    meta = {"haspos": haspos, "nsame": nsame, "fp": fp}
    return in_maps, meta


def _assemble(results, meta):
    """Combine per-core partials into the scalar loss (O(N) host math)."""
    haspos = meta["haspos"]
    nsame = meta["nsame"]

    neg = np.zeros(N, dtype=np.float64)   # (T1 - T2) per row
    posd = np.zeros(N, dtype=np.float64)  # <e_i, e_fp(i)>
    for c in range(NCORES):
        r = results[c]
        rows = np.arange(c * R, (c + 1) * R)
        ro = np.asarray(r["rowout"], np.float64)
        neg[rows] += ro[:, 0:MI].T.reshape(-1)
        posd[rows] += ro[:, MI:2 * MI].T.reshape(-1)
        cso = np.asarray(r["csout"], np.float64)      # [6, JW]
        for d in range(1, 4):
            b = (c + d) % NCORES
            rows_b = np.arange(b * R, b * R + R)
            # partition d-1 holds exp colsums, 3+d-1 the masked colsums of
            # the distance-d block; JW == R so they map 1:1 onto b's rows
            neg[rows_b] += cso[d - 1, :] - cso[3 + d - 1, :]

    posS = (posd + 1.0) * 0.25
    nneg = N - nsame
    total = neg + np.where(haspos, np.exp(posS), 1.0) + (2 * N - 2 - nneg)
    posval = np.where(haspos, posS, 0.0)
    loss = float(np.mean(np.log(total) - posval))
    return np.float32(loss)


def _install_ntff_shim():
    """Provide antenv.axon_hooks (absent in this image) so trace=True works."""
    import importlib
    import types
    try:
        importlib.import_module("antenv.axon_hooks")
        return
    except ImportError:
        pass
    try:
        import antenv
        from trn_agent_boot.trn_boot import _ntff_profile_via_ctypes

        hook = _ntff_profile_via_ctypes("/opt/axon/libaxon_pjrt.so")
        mod = types.ModuleType("antenv.axon_hooks")
        mod._hook = hook
        mod.get_axon_ntff_profile_hook = lambda: mod._hook
        mod.set_axon_ntff_profile_hook = lambda h: setattr(mod, "_hook", h)
        sys.modules["antenv.axon_hooks"] = mod
        antenv.axon_hooks = mod
    except Exception as e:  # profiling is best-effort
        print(f"ntff shim failed: {e}")


def kernel(layer_embeds, y_true, _trace=False):
    import time

    if _trace:
        _install_ntff_shim()
    nc = _get_program()
    in_maps, meta = _host_prep(layer_embeds, y_true)
    last_err = None
    for attempt in range(4):
        try:
            res = run_bass_kernel_spmd(
                nc, in_maps, core_ids=list(range(NCORES)), trace=_trace,
            )
            loss = _assemble(res.results, meta)
            # lse is bounded by log(2N-2) .. log(2N + N*e^0.5) for this
            # problem shape; anything outside is transient corruption.
            if not (np.isfinite(loss) and 5.0 < float(loss) < 20.0):
                raise RuntimeError(f"implausible loss {loss}, retrying")
            if _trace:
                return loss, res
            return loss
        except Exception as e:  # transient device faults: retry
            last_err = e
            time.sleep(5 * (attempt + 1))
    raise last_err


# revision 11
# speedup vs baseline: 1.0977x; 1.0977x over previous
"""Trainium2 Bass kernel for the contrastive loss problem.

Math reformulation of the reference (no [N, 2N-1] scatter needed):
  lse_i = log( exp(pos_val_i) + sum_{j in neg} exp(S_ij) + (2N-2-num_neg_i) )
  loss  = mean_i (lse_i - pos_val_i)
with S = (cos + 1) * 0.25, cos from row-normalized embeddings.

Sharding uses the Gram matrix's symmetry: core c computes only the
[512, 512*5] strip of exp(S) pairing its rows with block-columns
{c, c+1, .., c+4} (mod 8). Columns are pre-rotated on the host so the
program is identical on every core (SPMD). Row sums cover the strip;
one-hot-weight matmuls produce per-column sums for the foreign blocks
(distance 1..3), which the host adds to those rows' totals. Distance-4
blocks are computed by both endpoint cores (row sums only). The main
matmul runs in fp8 e4m3 (DoubleRow, K=256 per op) on x16-prescaled unit
rows. exp/masking on ScalarE/VectorE per 512-wide chunk; exp output is
stored fp8 and stacked across m-chunk pairs so the column-sum matmuls
are DoubleRow too (12 ops total, one PSUM bank, partitions 0:6).
Input DMAs are split across both HW DGE queues (sync + scalar) to
halve delivery latency; pos-pair row dots run on the idle Pool engine.

Host: norms, fp8/bf16 casts, rotation, first-positive gather (label
metadata), final assembly of ~4096 scalars.
"""

import sys

sys.path.insert(0, "/opt/trn_rl_repo")

from contextlib import ExitStack

import ml_dtypes
import numpy as np

import concourse.bacc as bacc
import concourse.tile as tile
from concourse import mybir
from concourse.bass_utils import run_bass_kernel_spmd

N, D = 4096, 1024
NCORES = 8
R = N // NCORES            # 512 rows per core
P = 128                    # partitions
MI = R // P                # 4 row chunks per core
KC = D // P                # 8 contraction chunks
JW = 512                   # j tile width (one PSUM bank)
NB = 5                     # block-columns per core (self + 4 right neighbors)
JCOLS = NB * JW            # 2560
EPS = 1e-8
BF16 = ml_dtypes.bfloat16
FP8 = ml_dtypes.float8_e4m3
SCALE = 16.0

_CACHE = {}


def _build_program():
    nc = bacc.Bacc("TRN2", target_bir_lowering=False, debug=False)
    f32, bf16, fp8 = mybir.dt.float32, mybir.dt.bfloat16, mybir.dt.float8e4
    AF = mybir.ActivationFunctionType
    OP = mybir.AluOpType
    DR = mybir.MatmulPerfMode.DoubleRow

    et_d = nc.dram_tensor("et", [KC, P, JCOLS], fp8, kind="ExternalInput")
    yt_d = nc.dram_tensor("yt", [P, JCOLS], bf16, kind="ExternalInput")
    yb_d = nc.dram_tensor("yb", [P, MI], f32, kind="ExternalInput")
    enef_d = nc.dram_tensor("enef", [MI, P, 2 * D], bf16, kind="ExternalInput")
    ro_d = nc.dram_tensor("rowout", [P, 2 * MI], f32, kind="ExternalOutput")
    cs_d = nc.dram_tensor("csout", [6, JW], f32, kind="ExternalOutput")

    with tile.TileContext(nc) as tc, ExitStack() as ctx:
        const = ctx.enter_context(tc.tile_pool(name="const", bufs=1))
        psum = ctx.enter_context(tc.tile_pool(name="psum", bufs=5, space="PSUM"))
        cspsum = ctx.enter_context(tc.tile_pool(name="cspsum", bufs=1,
                                                space="PSUM"))
        esp = ctx.enter_context(tc.tile_pool(name="esp", bufs=2))
        work = ctx.enter_context(tc.tile_pool(name="work", bufs=2))
        acc = ctx.enter_context(tc.tile_pool(name="acc", bufs=2))

        et = const.tile([P, KC, JCOLS], fp8, tag="et")
        yt = const.tile([P, JCOLS], bf16, tag="yt")
        yb = const.tile([P, MI], f32, tag="yb")
        enef = const.tile([P, MI, 2, D], bf16, tag="enef")
        b025 = const.tile([P, 1], f32, tag="b025")
        nc.vector.memset(b025, 0.25)
        ones = const.tile([P, 1], bf16, tag="ones")
        nc.gpsimd.memset(ones, 1.0)
        wsrc = const.tile([P, P], bf16, tag="wsrc")
        nc.gpsimd.memset(wsrc, 1.0)
        # one-hot DoubleRow weights: wcs[d][:, :, 0:6] has a 1 in column d.
        # 16-wide padding keeps the k-pair stride a multiple of 16 (DR ISA).
        wcs = [const.tile([P, 2, 16], fp8, tag=f"wc{d}", name=f"wc{d}")
               for d in range(6)]
        for d in range(6):
            nc.gpsimd.memset(wcs[d], 0.0)
            nc.gpsimd.memset(wcs[d][:, :, d:d + 1], 1.0)
        rowout = const.tile([P, 2 * MI], f32, tag="rowout")
        nsout = rowout[:, 0:MI]
        pdout = rowout[:, MI:2 * MI]
        cs = cspsum.tile([P, JW], f32, tag="cs")

        def warmup(n):
            # keep the PE clock hot while waiting on input DMAs: tiny
            # matmuls into a partition strip the column sums never touch
            for _ in range(n):
                nc.tensor.matmul(
                    cs[96:97, 0:P], ones, wsrc, start=True, stop=True,
                    tile_position=(0, 96), skip_group_check=True,
                )

        # Input DMAs all on the sync HW DGE queue (HBM bandwidth is the
        # shared cap; a second queue doesn't speed delivery but lengthens
        # the teardown). k-chunks first, in consumption order.
        for k in range(KC):
            nc.sync.dma_start(out=et[:, k, :], in_=et_d[k])
        nc.sync.dma_start(out=yt, in_=yt_d[:])
        nc.sync.dma_start(out=yb, in_=yb_d[:])
        for m in range(MI):
            nc.sync.dma_start(out=enef[:, m, :, :], in_=enef_d[m])
        # load the Exp table while waiting on DMAs
        warm = const.tile([P, 1], f32, tag="warm")
        nc.scalar.activation(warm, b025, AF.Exp, bias=b025, scale=1.0)

        for mp in range(2):
            es = [esp.tile([P, 2, JW], fp8, tag=f"es{j}", name=f"es{j}")
                  for j in range(NB)]
            mm = [esp.tile([P, 2, JW], fp8, tag=f"mm{j}", name=f"mm{j}")
                  for j in range(NB)]
            for mh in range(2):
                m = 2 * mp + mh

                def expmask(esj, mmj, pt, j, m, t1, t2, mh=0):
                    # expS = exp(cos*0.25 + 0.25); t1[:, j] = row-sum
                    nc.scalar.activation(
                        esj[:, mh, :], pt, AF.Exp, bias=b025,
                        scale=0.25 / (SCALE * SCALE),
                        accum_out=t1[:, j:j + 1],
                    )
                    # t2[:, j] = row-sum((y == y_row) * expS)
                    nc.vector.scalar_tensor_tensor(
                        mmj[:, mh, :], yt[:, j * JW:(j + 1) * JW],
                        yb[:, m:m + 1], esj[:, mh, :],
                        op0=OP.is_equal, op1=OP.mult,
                        accum_out=t2[:, j:j + 1],
                    )

                t1 = acc.tile([P, NB], f32, tag="t1")
                t2 = acc.tile([P, NB], f32, tag="t2")
                if m == 0:
                    # k2-outer: consume et chunk-pairs as they land, with
                    # warmup matmuls bridging the delivery gaps so the PE
                    # clock stays hot.
                    warmup(10)
                    pts = [psum.tile([P, JW], f32, tag="pt", name=f"pt{j}")
                           for j in range(NB)]
                    for k2 in range(KC // 2):
                        for j in range(NB):
                            nc.tensor.matmul(
                                pts[j],
                                et[:, 2 * k2:2 * k2 + 2, m * P:(m + 1) * P],
                                et[:, 2 * k2:2 * k2 + 2,
                                   j * JW:(j + 1) * JW],
                                start=(k2 == 0),
                                stop=(k2 == KC // 2 - 1),
                                perf_mode=DR,
                            )
                        if k2 < KC // 2 - 1:
                            warmup(6)
                    for j in range(NB):
                        expmask(es[j], mm[j], pts[j], j, m, t1, t2, mh)
                else:
                    for j in range(NB):
                        pt = psum.tile([P, JW], f32, tag="pt")
                        for k2 in range(KC // 2):
                            nc.tensor.matmul(
                                pt,
                                et[:, 2 * k2:2 * k2 + 2, m * P:(m + 1) * P],
                                et[:, 2 * k2:2 * k2 + 2,
                                   j * JW:(j + 1) * JW],
                                start=(k2 == 0),
                                stop=(k2 == KC // 2 - 1),
                                perf_mode=DR,
                            )
                        expmask(es[j], mm[j], pt, j, m, t1, t2, mh)
                # nsout[:, m] = sum_j(t1 - t2)
                d5 = acc.tile([P, NB], f32, tag="d5")
                nc.vector.scalar_tensor_tensor(
                    d5, t1, 1.0, t2, op0=OP.mult, op1=OP.subtract,
                    accum_out=nsout[:, m:m + 1],
                )
                # pdout[:, mq] = row-wise <e_i, e_firstpos(i)>; m=3's dot is
                # hoisted into m=2's slot so the tail isn't gated on it
                for mq in ([m] if m < 2 else [2, 3] if m == 2 else []):
                    pdo = work.tile([P, D], bf16, tag="pdo")
                    nc.vector.scalar_tensor_tensor(
                        pdo, enef[:, mq, 0, :], 1.0, enef[:, mq, 1, :],
                        op0=OP.mult, op1=OP.mult,
                        accum_out=pdout[:, mq:mq + 1],
                    )
            # column sums for the foreign blocks: DoubleRow over the stacked
            # m-chunk pair, one-hot weights land block d's sums in PSUM
            # partition d (es sums rows 0:3, mm sums rows 3:6).
            for jj in range(1, 4):
                nc.tensor.matmul(
                    cs[0:6, :], wcs[jj - 1][:, :, 0:6], es[jj],
                    start=(mp == 0 and jj == 1), stop=False, perf_mode=DR,
                )
            for jj in range(1, 4):
                nc.tensor.matmul(
                    cs[0:6, :], wcs[3 + jj - 1][:, :, 0:6], mm[jj],
                    start=False, stop=(mp == 1 and jj == 3), perf_mode=DR,
                )
        # evict column sums (DMA cannot read PSUM)
        csev = const.tile([P, JW], f32, tag="csev")
        nc.scalar.copy(csev[0:6, :], cs[0:6, :])
        nc.sync.dma_start(out=ro_d[:, :], in_=rowout)
        nc.sync.dma_start(out=cs_d[:, :], in_=csev[0:6, :])

    nc.compile()
    return nc


def _get_program():
    if "nc" not in _CACHE:
        _CACHE["nc"] = _build_program()
    return _CACHE["nc"]


def _host_prep(layer_embeds, y_true):
    E = np.asarray(layer_embeds, dtype=np.float32)
    y = np.asarray(y_true).astype(np.int32)

    norms = np.maximum(np.linalg.norm(E, axis=1), EPS).astype(np.float32)
    Ehf = E / norms[:, None]
    Eh = Ehf.astype(BF16)
    Eh8T = np.ascontiguousarray((Ehf * SCALE).astype(FP8).T)  # [D, N]

    same = y[:, None] == y[None, :]
    nsame = same.sum(1)
    haspos = nsame > 1
    np.fill_diagonal(same, False)
    fp = np.argmax(same, axis=1)                      # first positive (j order)
    yb16 = y.astype(BF16)

    in_maps = []
    for c in range(NCORES):
        r0, r1 = c * R, (c + 1) * R
        cols = np.concatenate(
            [np.arange(((c + b) % NCORES) * R, ((c + b) % NCORES) * R + R)
             for b in range(NB)])
        etc = np.ascontiguousarray(Eh8T[:, cols]).reshape(KC, P, JCOLS)
        ytc = np.ascontiguousarray(
            np.broadcast_to(yb16[cols][None, :], (P, JCOLS)))
        enc = Eh[r0:r1].reshape(MI, P, D)
        efc = Eh[fp[r0:r1]].reshape(MI, P, D)
        in_maps.append({
            "et": etc,
            "yt": ytc,
            "yb": np.ascontiguousarray(y[r0:r1].astype(np.float32)
                                       .reshape(MI, P).T),
            "enef": np.ascontiguousarray(
                np.concatenate([enc, efc], axis=2)),
        })
    meta = {"haspos": haspos, "nsame": nsame, "fp": fp}
    return in_maps, meta


def _assemble(results, meta):
    """Combine per-core partials into the scalar loss (O(N) host math)."""
    haspos = meta["haspos"]
    nsame = meta["nsame"]

    neg = np.zeros(N, dtype=np.float64)   # (T1 - T2) per row
    posd = np.zeros(N, dtype=np.float64)  # <e_i, e_fp(i)>
    for c in range(NCORES):
        r = results[c]
        rows = np.arange(c * R, (c + 1) * R)
        ro = np.asarray(r["rowout"], np.float64)
        neg[rows] += ro[:, 0:MI].T.reshape(-1)
        posd[rows] += ro[:, MI:2 * MI].T.reshape(-1)
        cso = np.asarray(r["csout"], np.float64)      # [6, JW]
        for d in range(1, 4):
            b = (c + d) % NCORES
            rows_b = np.arange(b * R, b * R + R)
            # partition d-1 holds exp colsums, 3+d-1 the masked colsums of
            # the distance-d block; JW == R so they map 1:1 onto b's rows
            neg[rows_b] += cso[d - 1, :] - cso[3 + d - 1, :]

    posS = (posd + 1.0) * 0.25
    nneg = N - nsame
    total = neg + np.where(haspos, np.exp(posS), 1.0) + (2 * N - 2 - nneg)
    posval = np.where(haspos, posS, 0.0)
    loss = float(np.mean(np.log(total) - posval))
    return np.float32(loss)


def _install_ntff_shim():
    """Provide antenv.axon_hooks (absent in this image) so trace=True works."""
    import importlib
    import types
    try:
        importlib.import_module("antenv.axon_hooks")
        return
    except ImportError:
        pass
    try:
        import antenv
        from trn_agent_boot.trn_boot import _ntff_profile_via_ctypes

        hook = _ntff_profile_via_ctypes("/opt/axon/libaxon_pjrt.so")
        mod = types.ModuleType("antenv.axon_hooks")
        mod._hook = hook
        mod.get_axon_ntff_profile_hook = lambda: mod._hook
        mod.set_axon_ntff_profile_hook = lambda h: setattr(mod, "_hook", h)
        sys.modules["antenv.axon_hooks"] = mod
        antenv.axon_hooks = mod
    except Exception as e:  # profiling is best-effort
        print(f"ntff shim failed: {e}")


def kernel(layer_embeds, y_true, _trace=False):
    import time

    if _trace:
        _install_ntff_shim()
    nc = _get_program()
    in_maps, meta = _host_prep(layer_embeds, y_true)
    last_err = None
    for attempt in range(4):
        try:
            res = run_bass_kernel_spmd(
                nc, in_maps, core_ids=list(range(NCORES)), trace=_trace,
            )
            loss = _assemble(res.results, meta)
            # lse is bounded by log(2N-2) .. log(2N + N*e^0.5) for this
            # problem shape; anything outside is transient corruption.
            if not (np.isfinite(loss) and 5.0 < float(loss) < 20.0):
                raise RuntimeError(f"implausible loss {loss}, retrying")
            if _trace:
                return loss, res
            return loss
        except Exception as e:  # transient device faults: retry
            last_err = e
            time.sleep(5 * (attempt + 1))
    raise last_err
